# revision 27
# baseline (speedup 1.0000x reference)
"""JointEdgeSegLoss Trainium2 kernel, v5.2: predicated-gather + max-telescope.

Per core (294912 pixels as [128, 2304], 8 chunks of 288 in 4 pair-groups):
  - Upfront full-row DMAs of t/e/m; tpk = t + 32*(e>0.8) in one STT; both
    halves' count scans (tensor_scalar is_equal accum, 4x) run immediately,
    filling DVE while the segin stream arrives.
  - Per group: ACT exp -> eb bf16; Pool builds the 19 class masks and the
    first tree level; DVE finishes sum_C, ACT Ln -> lse; 19 copy_predicated
    gather exp(x[t]) into sel; ACT Ln(sel) = x[t].
  - Per half: z = tpk + (x[t]-lse)/17 in fp16; 40-threshold max-telescope
    (tensor_scalar op0=max accum, 4x) gives per-class lp sums on the host
    via G(th)-G(th+1) + cumulative counts. BCE partials per half.
  - Host combines per-core [P, SLOTS] partials in float64.

Self-contained: hardcodes all shapes; only imports the runtime (concourse).
"""

import numpy as np

import concourse.bass as bass
import concourse.bacc as bacc
import concourse.mybir as mybir
import concourse.tile as tile
from concourse import bass_utils

F32 = mybir.dt.float32
U8 = mybir.dt.uint8
I32 = mybir.dt.int32
BF16 = mybir.dt.bfloat16
FP16 = mybir.dt.float16
ALU = mybir.AluOpType
ACTF = mybir.ActivationFunctionType

C = 19
N, H, W = 4, 768, 768
HW = H * W
NCORES = 8
P = 128
Q = 2304                         # columns per partition per core
F = 288                          # columns per chunk
NCH = Q // F                     # 8 chunks
NG = 4                           # pair-groups
FG = Q // NG                     # 576 columns per group
QH = Q // 2                      # half width (telescope/count granularity)
EDGE_THRESH = 0.8
BKEY = 32.0                      # tpack = t + 32*gt
VSCALE = 17.0                    # z frac = lp / VSCALE, lp in (-16, 0)

# scan blocks: [0:1152] (groups 0-1), [1152:1728] (g2), [1728:2304] (g3)
BLOCKS = [(0, 1152), (1152, 2304)]
NBLK = len(BLOCKS)
# slots: G-telescope NBLK x 40, counts NBLK x 38, bce 2 halves x 5
NGT = 40
SL_G = 0
SL_N = NBLK * NGT
SL_BCE = SL_N + NBLK * 38
SLOTS = SL_BCE + 10              # 234

# telescope thresholds, indexed j=0..39:
#   A: th = j - 1 for j in 0..19 ; B: th = 31 + (j - 20) for j in 20..39
THRESH = [float(j - 1) for j in range(20)] + [float(31 + j) for j in range(20)]


def build_program():
    nc = bacc.Bacc("TRN2", target_bir_lowering=False, debug=False)

    xs = nc.dram_tensor("xs", [C, P, Q], F32, kind="ExternalInput")
    ts = nc.dram_tensor("ts", [P, Q], I32, kind="ExternalInput")
    es = nc.dram_tensor("es", [P, Q], F32, kind="ExternalInput")
    ms = nc.dram_tensor("ms", [P, Q], I32, kind="ExternalInput")
    acc_d = nc.dram_tensor("acc", [P, SLOTS], F32, kind="ExternalOutput")

    with tile.TileContext(nc) as tc:
        with (
            tc.tile_pool(name="xp", bufs=2) as xp,
            tc.tile_pool(name="ep", bufs=2) as ep,
            tc.tile_pool(name="sp", bufs=2) as sp,
            tc.tile_pool(name="cst", bufs=1) as cst,
        ):
            accT = cst.tile([P, SLOTS], F32, tag="acc")
            Tf = cst.tile([P, Q], I32, tag="Tf")
            Ef = cst.tile([P, Q], F32, tag="Ef")
            t_h = cst.tile([P, Q], FP16, tag="t_h")
            tpk = cst.tile([P, Q], FP16, tag="tpk")
            lseb = cst.tile([P, Q], BF16, tag="lseb")
            sel = cst.tile([P, Q], BF16, tag="sel")
            selp = cst.tile([P, Q], BF16, tag="selp")  # also hosts gt early
            mkB0 = cst.tile([P, C, FG], U8, tag="mkB0")
            mkB1 = cst.tile([P, C, FG], U8, tag="mkB1")
            mkBs = [mkB0, mkB1]
            junkH = cst.tile([P, QH], FP16, tag="junkH")
            zbh = cst.tile([P, QH], BF16, tag="zbh")
            zfh = cst.tile([P, QH], FP16, tag="zfh")
            t9 = cst.tile([P, 9, FG], BF16, tag="t9")
            t4 = cst.tile([P, 4, FG], BF16, tag="t4")
            t2 = cst.tile([P, 2, FG], BF16, tag="t2")
            t1 = cst.tile([P, FG], BF16, tag="t1")
            t1b = cst.tile([P, FG], BF16, tag="t1b")
            bA = cst.tile([P, QH], BF16, tag="bA")
            bB = cst.tile([P, QH], BF16, tag="bB")
            bC = cst.tile([P, QH], BF16, tag="bC")

            def slot(j):
                return accT[:, j:j + 1]

            # ---- upfront small DMAs + packed key + early count scans ----
            nc.sync.dma_start(Ef[:, :QH], es.ap()[:, :QH])
            nc.sync.dma_start(Tf[:], ts.ap()[:, :])
            nc.sync.dma_start(Ef[:, QH:], es.ap()[:, QH:])

            gt = selp  # selp buffer is free until the gather phase
            for h in range(2):
                hsl = slice(h * QH, (h + 1) * QH)
                nc.vector.tensor_scalar(
                    gt[:, hsl], Ef[:, hsl], EDGE_THRESH, None, op0=ALU.is_gt)
                nc.vector.scalar_tensor_tensor(
                    tpk[:, hsl], gt[:, hsl], BKEY, Tf[:, hsl],
                    op0=ALU.mult, op1=ALU.add)
            nc.gpsimd.tensor_copy(t_h[:], Tf[:])
            Mf = Tf  # T is dead once t_h/tpk exist; reuse its buffer for m

            def emit_counts(b):
                b0, b1 = BLOCKS[b]
                nb = SL_N + b * 38
                for c in range(2 * C):
                    key = float(c % C) + (BKEY if c >= C else 0.0)
                    nc.vector.tensor_scalar(
                        junkH[:, :b1 - b0], tpk[:, b0:b1], key, None,
                        op0=ALU.is_equal, op1=ALU.add,
                        accum_out=slot(nb + c))

            emit_counts(0)

            # ---- main stream ----
            for g in range(NG):
                gsl = slice(g * FG, (g + 1) * FG)
                ebP = ep.tile([P, C, FG], BF16, tag="eb")

                for u in range(2):
                    k = 2 * g + u
                    sl = slice(k * F, (k + 1) * F)
                    X = xp.tile([P, C, F], F32, tag="X")
                    nc.sync.dma_start(
                        X[:], xs.ap()[:, :, sl].transpose([1, 0, 2]))
                    nc.scalar.activation(
                        ebP[:, :, u * F:(u + 1) * F], X[:], ACTF.Exp)
                if g == 0:
                    # edgemask only needed for the bce dot-products
                    nc.sync.dma_start(Mf[:], ms.ap()[:, :])

                # all 19 masks for this group on Pool (runs ahead of DVE)
                mkB = mkBs[g % 2]
                for c in range(1, C):
                    nc.gpsimd.tensor_scalar(
                        mkB[:, c, :], t_h[:, gsl], float(c), None,
                        op0=ALU.is_equal)
                # tree level 1: Pool, except last group (tail) on DVE
                eng1 = nc.vector
                eng1.tensor_tensor(
                    t9[:], ebP[:, 0:9, :], ebP[:, 9:18, :], op=ALU.add)

                # rest of sum over C on DVE
                sG = sp.tile([P, FG], BF16, tag="sG")
                nc.vector.tensor_tensor(
                    t4[:], t9[:, 0:4, :], t9[:, 4:8, :], op=ALU.add)
                nc.vector.tensor_tensor(
                    t2[:], t4[:, 0:2, :], t4[:, 2:4, :], op=ALU.add)
                nc.vector.tensor_tensor(
                    t1[:], t2[:, 0, :], t2[:, 1, :], op=ALU.add)
                nc.vector.tensor_tensor(
                    t1b[:], t1[:], t9[:, 8, :], op=ALU.add)
                nc.vector.tensor_tensor(
                    sG[:], t1b[:], ebP[:, 18, :], op=ALU.add)
                nc.scalar.activation(lseb[:, gsl], sG[:], ACTF.Ln)
                if g == 0:
                    emit_counts(1)  # fills the wait for masks/lse

                # gather exp(x[t]): class 0 by plain copy (2x mode),
                # the rest by predicated copies over it
                nc.vector.tensor_copy(sel[:, gsl], ebP[:, 0, :])
                for c in range(1, C):
                    nc.vector.copy_predicated(
                        sel[:, gsl], mkB[:, c, :], ebP[:, c, :])
                # x[t] = ln(sel)
                nc.scalar.activation(selp[:, gsl], sel[:, gsl], ACTF.Ln)

                # ---- per-block telescope (blocks end at groups 1,3) ----
                if g in (1, 3):
                    b = g // 2
                    b0, b1 = BLOCKS[b]
                    w = b1 - b0
                    nc.vector.tensor_tensor(
                        zbh[:, :w], selp[:, b0:b1], lseb[:, b0:b1],
                        op=ALU.subtract)
                    nc.vector.scalar_tensor_tensor(
                        zfh[:, :w], zbh[:, :w], 1.0 / VSCALE, tpk[:, b0:b1],
                        op0=ALU.mult, op1=ALU.add)
                    for j, th in enumerate(THRESH):
                        nc.vector.tensor_scalar(
                            junkH[:, :w], zfh[:, :w], th, None,
                            op0=ALU.max, op1=ALU.add,
                            accum_out=slot(SL_G + b * NGT + j))

                # ---- bce per half, early (only needs Ef/Mf) ----
                if g in (1, 3):
                    h = g // 2
                    hsl = slice(h * QH, (h + 1) * QH)
                    bs = SL_BCE + 5 * h
                    nc.scalar.activation(bA[:], Ef[:, hsl], ACTF.Abs)
                    nc.scalar.activation(bB[:], bA[:], ACTF.Exp, scale=-1.0)
                    nc.scalar.activation(bC[:], Ef[:, hsl], ACTF.Relu,
                                         accum_out=slot(bs + 3))
                    nc.scalar.activation(bA[:], bB[:], ACTF.Ln, bias=1.0,
                                         accum_out=slot(bs + 4))
                    nc.vector.scalar_tensor_tensor(
                        bB[:], Mf[:, hsl], 1.0, Ef[:, hsl],
                        op0=ALU.is_equal, op1=ALU.mult,
                        accum_out=slot(bs + 0))
                    nc.vector.tensor_tensor(
                        bB[:], bC[:], bA[:], op=ALU.add)
                    nc.vector.scalar_tensor_tensor(
                        bC[:], Mf[:, hsl], 1.0, bB[:],
                        op0=ALU.is_equal, op1=ALU.mult,
                        accum_out=slot(bs + 1))
                    # zbh is dead after zf; reuse as bf16 copy of m for
                    # the sum-m scan (tensor_scalar rejects i32 inputs)
                    nc.gpsimd.tensor_copy(zbh[:], Mf[:, hsl])
                    nc.vector.tensor_scalar(
                        bA[:], zbh[:], 1.0, None,
                        op0=ALU.is_equal, op1=ALU.add,
                        accum_out=slot(bs + 2))


            nc.sync.dma_start(acc_d.ap()[:, :], accT[:])

    nc.finalize()
    return nc


_CACHE = {}


def _get_program():
    if "nc" not in _CACHE:
        _CACHE["nc"] = build_program()
    return _CACHE["nc"]


def make_in_maps(segin, edgein, segmask, edgemask):
    in_maps = []
    for k in range(NCORES):
        n, h = k // 2, k % 2
        rs = slice(h * (H // 2), (h + 1) * (H // 2))
        in_maps.append({
            "xs": np.ascontiguousarray(
                segin[n, :, rs, :].reshape(C, P, Q)),
            "ts": np.ascontiguousarray(
                segmask[n, rs, :].reshape(P, Q)),
            "es": np.ascontiguousarray(
                edgein[n, 0, rs, :].reshape(P, Q)),
            "ms": np.ascontiguousarray(
                edgemask[n, 0, rs, :].reshape(P, Q)),
        })
    return in_maps


def core_quants(part):
    """part: [SLOTS] f64 sums for one core -> (SA, SB, NA, NB, bce[5]).

    SA/SB are per-class sums of lp = x[t]-lse over (t==c & ~gt) and
    (t==c & gt), reconstructed from the max-telescope:
      sum_{key==K} lp/17 = G(K-1) - G(K) + N_{key <= K-1}
    with per-half G scans and per-half cumulative counts; B-side
    cumulatives include all A-pixels (their z < 19 < 31).
    """
    SA = np.zeros(C)
    SB = np.zeros(C)
    NA = np.zeros(C)
    NB = np.zeros(C)
    for h in range(NBLK):
        G = part[SL_G + h * NGT: SL_G + (h + 1) * NGT]
        NAh = part[SL_N + h * 38: SL_N + h * 38 + 19]
        NBh = part[SL_N + h * 38 + 19: SL_N + h * 38 + 38]
        NA = NA + NAh
        NB = NB + NBh
        cumA = np.concatenate([[0.0], np.cumsum(NAh)])
        cumB = np.concatenate([[0.0], np.cumsum(NBh)])
        na_tot = NAh.sum()
        for c in range(C):
            SA[c] += VSCALE * (G[c] - G[c + 1] + cumA[c])
            SB[c] += VSCALE * (G[20 + c] - G[21 + c] + na_tot + cumB[c])
    return SA, SB, NA, NB, part[SL_BCE:SL_BCE + 5] + part[SL_BCE + 5:SL_BCE + 10]


def combine(acc_list):
    """acc_list: per-core [P, SLOTS] arrays -> final f32 scalar loss."""
    part = np.zeros((NCORES, SLOTS))
    for k in range(NCORES):
        part[k] = acc_list[k].astype(np.float64).sum(axis=0)

    q = [core_quants(part[k]) for k in range(NCORES)]

    seg_loss = 0.0
    att_loss = 0.0
    for n in range(N):
        c0, c1 = 2 * n, 2 * n + 1
        S1 = q[c0][0] + q[c0][1] + q[c1][0] + q[c1][1]
        S2 = q[c0][1] + q[c1][1]
        bins = q[c0][2] + q[c0][3] + q[c1][2] + q[c1][3]
        bins2 = q[c0][3] + q[c1][3]

        w1 = (bins != 0) * (1.0 - bins / HW) + 1.0
        seg_loss += -(w1 * S1).sum() / (w1 * bins).sum()

        vsum = bins2.sum()
        w2 = (bins2 != 0) * (1.0 - bins2 / vsum) + 1.0
        att_loss += -(w2 * S2).sum() / (w2 * bins2).sum()

    bce = sum(qq[4] for qq in q)
    sum_em, sum_b1m, pos_num, sum_relu, sum_l1p = bce

    all_bce = sum_relu + sum_l1p - sum_em
    pos_bce = sum_b1m - sum_em
    cnt = float(N * HW)
    neg_num = cnt - pos_num
    neg_bce = all_bce - pos_bce
    ssum = pos_num + neg_num
    edge_loss = (neg_num / ssum * pos_bce + pos_num / ssum * neg_bce) / cnt

    return np.float32(seg_loss + 0.3 * edge_loss + 0.1 * att_loss)


def run_cores(in_maps, trace=False, **kw):
    nc = _get_program()
    res = bass_utils.run_bass_kernel_spmd(
        nc, in_maps, core_ids=list(range(NCORES)), trace=trace, **kw
    )
    return res


def kernel(segin, edgein, segmask, edgemask):
    in_maps = make_in_maps(
        np.asarray(segin), np.asarray(edgein),
        np.asarray(segmask), np.asarray(edgemask))
    res = run_cores(in_maps)
    acc_list = [out["acc"] for out in res.results]
    return combine(acc_list)
